# revision 1
# baseline (speedup 1.0000x reference)
"""Block-causal attention Trainium2 kernel (8 NeuronCores).

Sharding: core c = b*4 + g handles batch b (of 2) and head-group g (4 of 16
heads). Each core computes the qkv projection, rmsnorm + 2-D RoPE,
block-causal attention and a partial output projection for its 256 channels;
the host sums the 4 per-group partials per batch.

On-chip layouts (per core):
  Q^T/K^T: feature-on-partition tiles QR/QI/KR/KI [128, 2048]; row 32*hh+j
    <-> head hh, complex pair j (R = even orig dim 2j, I = odd 2j+1).
  V: [l, d] tiles per head [128, 16, 65] with an all-ones column 64 so the
    softmax denominator falls out of the M=65 PV matmul.
  Scores: S^T [keys=128, q=256] per (head, frame, ktile); block-causal means
    frame t only attends keys < 256*(t+1) -- no mask tensor anywhere.
  rmsnorm: r = rsqrt(mean(q^2)+eps) via weighted ones-matmul over partitions;
    q_scale/k_scale are folded into the projection weights; the k-side
    0.125*r_k is folded into exp()'s per-partition scale and the q-side r_q
    is multiplied into Q^T during RoPE. exp() needs no max-subtraction
    (|scores| <= 8 after rmsnorm).
Matmuls run in bf16 (the fp32r path lowers to slow 2-pass fp32 on this HW).
"""

import os
import numpy as np

import concourse.bass as bass
import concourse.mybir as mybir
import concourse.tile as tile
from concourse import bacc
from concourse.bass_utils import run_bass_kernel_spmd

F32 = mybir.dt.float32
F32R = mybir.dt.float32r
BF16 = mybir.dt.bfloat16
AF = mybir.ActivationFunctionType
MUL = mybir.AluOpType.mult
ADD = mybir.AluOpType.add
SUB = mybir.AluOpType.subtract

B, T, NP, D, H = 2, 8, 256, 1024, 16
L = T * NP            # 2048
HD = 64               # head dim
HPG = 4               # heads per group (4 groups x 2 batches = 8 cores)
CPG = HPG * HD        # 256 channels per group
NDT = D // 128        # 8 d-tiles
NLC = L // 512        # 4 l-chunks
NLT = L // 128        # 16 l-tiles
EPS = 1e-6

_CACHE = {}


def _emit(nc, tc, ctx, xT, wqk, wv, wo, wvec, costab, sintab, out, skb):
    sing = ctx.enter_context(tc.tile_pool(name="sing", bufs=1))
    xp = ctx.enter_context(tc.tile_pool(name="xp", bufs=8))
    tmp = ctx.enter_context(tc.tile_pool(name="tmp", bufs=2))
    sqp = ctx.enter_context(tc.tile_pool(name="sqp", bufs=2))
    ptp = ctx.enter_context(tc.tile_pool(name="ptp", bufs=8))
    rbp = ctx.enter_context(tc.tile_pool(name="rbp", bufs=2))
    osb = ctx.enter_context(tc.tile_pool(name="osb", bufs=2))
    rcp = ctx.enter_context(tc.tile_pool(name="rcp", bufs=2))
    # PSUM: shared 1-bank pool (4) + 2-bank score pool (2x2) = 8 banks
    pps = ctx.enter_context(tc.tile_pool(name="pps", bufs=3, space="PSUM"))
    pst = ctx.enter_context(tc.tile_pool(name="pst", bufs=5, space="PSUM"))

    # ---- persistent SBUF ----
    wqk_sb = sing.tile([128, NDT, 512], BF16)
    nc.sync.dma_start(out=wqk_sb[:], in_=wqk.rearrange("(t p) o -> p t o", p=128))
    wv_sb = sing.tile([128, NDT, CPG], BF16)
    nc.sync.dma_start(out=wv_sb[:], in_=wv.rearrange("(t p) o -> p t o", p=128))
    wvec_sb = sing.tile([128, 4], BF16)
    nc.sync.dma_start(out=wvec_sb[:], in_=wvec[:])
    cos_sb = sing.tile([128, L], F32)
    nc.scalar.dma_start(out=cos_sb[:], in_=costab[:])
    sin_sb = sing.tile([128, L], F32)
    nc.scalar.dma_start(out=sin_sb[:], in_=sintab[:])
    wo_sb = sing.tile([128, 2, D], BF16)
    nc.scalar.dma_start(out=wo_sb[:], in_=wo.rearrange("(t p) o -> p t o", p=128))

    qk_sb = [sing.tile([128, L], BF16, name=f"qk{i}") for i in range(4)]
    rope_sb = [sing.tile([128, L], BF16, name=f"rope{i}") for i in range(4)]
    v_sb = [sing.tile([128, NLT, 65], BF16, name=f"v{h}") for h in range(HPG)]
    att_sb = [sing.tile([128, L], BF16, name=f"att{i}") for i in range(2)]
    ones_f32 = sing.tile([128, NLT, 1], F32)
    nc.vector.memset(ones_f32[:], 1.0)
    for h in range(HPG):
        nc.vector.tensor_copy(v_sb[h][:, :, 64:65], ones_f32[:])

    epsP = sing.tile([128, 1], F32)
    nc.vector.memset(epsP[:], EPS)
    eps64P = sing.tile([128, 1], F32)
    nc.vector.memset(eps64P[:], 64.0 * EPS)
    Rq = sing.tile([128, L], F32)
    Rn1 = sing.tile([128, L], F32)
    # per-lc shuffled, per-head-contiguous q/k tiles; K pre-scaled by 0.125*r_k
    QPl = [[sing.tile([128, 512], BF16, name=f"qp{i}_{c}") for c in range(NLC)]
           for i in range(2)]
    KPl = [[sing.tile([128, 512], BF16, name=f"kp{i}_{c}") for c in range(NLC)]
           for i in range(2)]

    # ---- attention groups (frame pairs, kt-major); callable for interleave ----
    def att_group(hp, fp):
        f0, f1 = 2 * fp, 2 * fp + 1
        nkt_sh, nkt_all = 4 * fp + 2, 4 * fp + 4
        pvps = [pst.tile([65, 512], F32, name=f"pv{hp}_{fp}_{i}", tag="st")
                for i in range(2)]
        pend = []

        def flush_pv():
            for kt_, i_, pt_ in pend:
                h_ = hp * 2 + i_
                if kt_ < nkt_sh:
                    nc.tensor.matmul(pvps[i_][:, :], v_sb[h_][:, kt_, :],
                                     pt_[:, 0:512], start=(kt_ == 0),
                                     stop=(kt_ == nkt_all - 1),
                                     skip_group_check=True)
                else:
                    nc.tensor.matmul(pvps[i_][:, 256:512], v_sb[h_][:, kt_, :],
                                     pt_[:, 0:256], start=False,
                                     stop=(kt_ == nkt_all - 1),
                                     skip_group_check=True)
            pend.clear()

        for kt in range(nkt_all):
            lck, kof = kt // 4, (kt % 4) * 128
            ksl = slice(kof, kof + 128)
            shared = kt < nkt_sh
            nq = 512 if shared else 256
            cur = []
            for i in range(2):
                qsrc = (QPl[hp][fp][64 * i:64 * i + 64, :] if shared else
                        QPl[hp][fp][64 * i:64 * i + 64, 256:512])
                st = pst.tile([128, nq], F32, name=f"st{i}{nq}", tag="st")
                nc.tensor.matmul(st[:, :],
                                 KPl[hp][lck][64 * i:64 * i + 64, ksl],
                                 qsrc, start=True, stop=True,
                                 skip_group_check=True)
                pt = ptp.tile([128, nq], BF16, name=f"pt{i}{nq}", tag="pt")
                nc.scalar.activation(pt[:], st[:], AF.Exp)
                cur.append((kt, i, pt))
            if len(pend) >= 4:
                flush_pv()
            pend.extend(cur)
        flush_pv()

        sstage = rbp.tile([1, 1024], F32, tag="sstage")
        for i in range(2):
            for fi in range(2):
                qs = slice((2 * fp + fi) * 256, (2 * fp + fi) * 256 + 256)
                cs = slice(fi * 256, fi * 256 + 256)
                dst = att_sb[hp][64 * i:64 * i + 64, qs]
                srow = sstage[:, (2 * i + fi) * 256:(2 * i + fi + 1) * 256]
                if (fi + i) % 2 == 0:
                    nc.vector.tensor_copy(dst, pvps[i][0:64, cs])
                    nc.scalar.activation(srow, pvps[i][64:65, cs], AF.Copy)
                else:
                    nc.scalar.activation(dst, pvps[i][0:64, cs], AF.Copy)
                    nc.vector.tensor_copy(srow, pvps[i][64:65, cs])
        nc.sync.dma_start(
            out=skb[8 + 2 * hp:10 + 2 * hp, 512 * fp:512 * fp + 512],
            in_=sstage[:])

    def normalize_hp(hp):
        Rn = Rq if hp == 0 else Rn1
        for i in range(2):
            h = hp * 2 + i
            nc.gpsimd.dma_start(out=Rn[64 * i:64 * i + 64, :],
                                in_=skb[8 + h:9 + h, :].to_broadcast((64, L)))
        nc.vector.reciprocal_approx_fast(out=Rn[:], in_=Rn[:])
        for c4 in range(4):
            cs = slice(c4 * 512, (c4 + 1) * 512)
            nc.vector.tensor_tensor(att_sb[hp][:, cs], att_sb[hp][:, cs],
                                    Rn[:, cs], MUL)


    # ---- phase 1: projections + rms sums + rope + shuffle, per l-chunk ----
    for lc in range(NLC):
        ls = slice(lc * 512, (lc + 1) * 512)
        xt = []
        for dt in range(NDT):
            x1 = xp.tile([128, 512], BF16, name=f"xt{dt}", tag="xt")
            nc.sync.dma_start(out=x1[:], in_=xT[dt * 128:(dt + 1) * 128, ls])
            xt.append(x1)
        for pair, rowbase, wcol in ((0, 0, 0), (2, 4, 2)):
            sqs = []
            for comp in range(2):           # R then I
                ot = pair + comp
                ps = pps.tile([128, 512], F32, name="qkps", tag="ps")
                for dt in range(NDT):
                    nc.tensor.matmul(ps[:], wqk_sb[:, dt, ot * 128:(ot + 1) * 128],
                                     xt[dt][:], start=(dt == 0), stop=(dt == NDT - 1))
                nc.vector.tensor_copy(qk_sb[ot][:, ls], ps[:])
                sq = sqp.tile([128, 512], BF16, tag="sq")
                nc.vector.tensor_tensor(sq[:], qk_sb[ot][:, ls],
                                        qk_sb[ot][:, ls], MUL)
                sqs.append(sq)
            for hh in range(HPG):
                r0 = 32 * hh
                rs = pps.tile([1, 512], F32, name="rmsps", tag="ps")
                nc.tensor.matmul(rs[:], wvec_sb[r0:r0 + 32, wcol:wcol + 1],
                                 sqs[0][r0:r0 + 32, :], start=True, stop=False,
                                 tile_position=(r0, 0), skip_group_check=True)
                nc.tensor.matmul(rs[:], wvec_sb[r0:r0 + 32, wcol + 1:wcol + 2],
                                 sqs[1][r0:r0 + 32, :], start=False, stop=True,
                                 tile_position=(r0, 0), skip_group_check=True)
                rrow = rcp.tile([1, 512], F32, tag="rrow")
                nc.vector.tensor_copy(rrow[:], rs[:])
                nc.gpsimd.dma_start(
                    out=skb[rowbase + hh:rowbase + hh + 1, ls], in_=rrow[:])
        # V projection: l on partitions
        for ls4 in range(4):
            lt = lc * 4 + ls4
            ps = pps.tile([128, CPG], F32, name="vps", tag="ps")
            for dt in range(NDT):
                nc.tensor.matmul(ps[:], xt[dt][:, ls4 * 128:(ls4 + 1) * 128],
                                 wv_sb[:, dt, :], start=(dt == 0),
                                 stop=(dt == NDT - 1))
            for h in range(HPG):
                nc.vector.tensor_copy(v_sb[h][:, lt, 0:64],
                                      ps[:, h * 64:(h + 1) * 64])

        if lc >= 1:
            att_group(0, lc - 1)
            att_group(1, lc - 1)

        # q-side r chain (per-lc)
        for h in range(HPG):
            nc.gpsimd.dma_start(out=Rq[32 * h:32 * h + 32, ls],
                                in_=skb[h:h + 1, ls].to_broadcast((32, 512)))
        nc.scalar.activation(Rq[:, ls], Rq[:, ls], AF.Sqrt, bias=epsP[:])
        nc.vector.reciprocal_approx_fast(out=Rq[:, ls], in_=Rq[:, ls])

        # per-lc RoPE (+ r_q fold on the q side)
        for base in (0, 2):
            xr, xi = qk_sb[base][:, ls], qk_sb[base + 1][:, ls]
            for comp in range(2):
                t1 = tmp.tile([128, 512], F32, tag="t1")
                t2 = tmp.tile([128, 512], F32, tag="t2")
                ca, cb = (cos_sb, sin_sb) if comp == 0 else (sin_sb, cos_sb)
                nc.vector.tensor_tensor(t1[:], xr, ca[:, ls], MUL)
                nc.vector.tensor_tensor(t2[:], xi, cb[:, ls], MUL)
                op = SUB if comp == 0 else ADD
                dst = rope_sb[base + comp][:, ls]
                if base == 0:
                    t3 = tmp.tile([128, 512], F32, tag="t3")
                    nc.vector.tensor_tensor(t3[:], t1[:], t2[:], op)
                    nc.vector.tensor_tensor(dst, t3[:], Rq[:, ls], MUL)
                else:
                    nc.vector.tensor_tensor(dst, t1[:], t2[:], op)

        # shuffle into per-head contiguous tiles (ACT hwdge queue)
        for hp2 in range(2):
            for i2 in range(2):
                h2 = hp2 * 2 + i2
                nc.scalar.dma_start(out=QPl[hp2][lc][64 * i2:64 * i2 + 32, :],
                                    in_=rope_sb[0][32 * h2:32 * h2 + 32, ls])
                nc.scalar.dma_start(out=QPl[hp2][lc][64 * i2 + 32:64 * i2 + 64, :],
                                    in_=rope_sb[1][32 * h2:32 * h2 + 32, ls])
                nc.scalar.dma_start(out=KPl[hp2][lc][64 * i2:64 * i2 + 32, :],
                                    in_=rope_sb[2][32 * h2:32 * h2 + 32, ls])
                nc.scalar.dma_start(out=KPl[hp2][lc][64 * i2 + 32:64 * i2 + 64, :],
                                    in_=rope_sb[3][32 * h2:32 * h2 + 32, ls])
        # K pre-scale by 0.125*r_k (per head rows), in place
        for hp2 in range(2):
            rkb = tmp.tile([128, 512], F32, tag="rkb")
            for i2 in range(2):
                h2 = hp2 * 2 + i2
                nc.gpsimd.dma_start(
                    out=rkb[64 * i2:64 * i2 + 64, :],
                    in_=skb[4 + h2:5 + h2, ls].to_broadcast((64, 512)))
            nc.scalar.activation(rkb[:], rkb[:], AF.Sqrt, bias=eps64P[:])
            nc.vector.reciprocal_approx_fast(out=rkb[:], in_=rkb[:])
            nc.vector.tensor_tensor(KPl[hp2][lc][:], KPl[hp2][lc][:], rkb[:], MUL)

    att_group(0, 3)
    normalize_hp(0)
    att_group(1, 3)
    normalize_hp(1)

    # ---- phase 5: output projection ----
    for lt in range(NLT):
        lsl = slice(lt * 128, (lt + 1) * 128)
        for oc in range(2):
            ps = pps.tile([128, 512], F32, name=f"op{lt}_{oc}", tag="ps")
            for ct in range(2):
                nc.tensor.matmul(ps[:], att_sb[ct][:, lsl],
                                 wo_sb[:, ct, oc * 512:(oc + 1) * 512],
                                 start=(ct == 0), stop=(ct == 1))
            ob = osb.tile([128, 512], F32, tag="ob")
            if (lt + oc) % 2 == 0:
                nc.vector.tensor_copy(ob[:], ps[:])
            else:
                nc.scalar.activation(ob[:], ps[:], AF.Copy)
            nc.sync.dma_start(out=out[lsl, oc * 512:(oc + 1) * 512], in_=ob[:])


def _build_nc():
    import contextlib
    nc = bacc.Bacc("TRN2", target_bir_lowering=False, debug=False, num_devices=8)
    xT = nc.dram_tensor("xT", (D, L), BF16, kind="ExternalInput")
    wqk = nc.dram_tensor("wqk", (D, 512), BF16, kind="ExternalInput")
    wv = nc.dram_tensor("wv", (D, CPG), BF16, kind="ExternalInput")
    wo = nc.dram_tensor("wo", (CPG, D), BF16, kind="ExternalInput")
    wvec = nc.dram_tensor("wvec", (128, 4), BF16, kind="ExternalInput")
    costab = nc.dram_tensor("costab", (128, L), F32, kind="ExternalInput")
    sintab = nc.dram_tensor("sintab", (128, L), F32, kind="ExternalInput")
    out = nc.dram_tensor("out", (L, D), F32, kind="ExternalOutput")
    skb = nc.dram_tensor("skb", (12, L), F32)

    with tile.TileContext(nc) as tc, contextlib.ExitStack() as ctx:
        _emit(nc, tc, ctx, xT.ap(), wqk.ap(), wv.ap(), wo.ap(), wvec.ap(),
              costab.ap(), sintab.ap(), out.ap(), skb.ap())
    nc.compile()
    return nc


def _host_prep(x, Wqkv, Wout, q_scale, k_scale):
    x = np.asarray(x, np.float32)
    Wqkv = np.asarray(Wqkv, np.float32)
    Wout = np.asarray(Wout, np.float32)
    q_scale = np.asarray(q_scale, np.float32)
    k_scale = np.asarray(k_scale, np.float32)

    quarter = HD // 4  # 16
    inv = 1.0 / (10000.0 ** (np.arange(quarter, dtype=np.float64) / quarter))
    tt = np.repeat(np.arange(T), NP).astype(np.float64)
    pp = np.tile(np.arange(NP), T).astype(np.float64)
    ang = np.concatenate([tt[:, None] * inv[None, :], pp[:, None] * inv[None, :]],
                         axis=1)  # (L, 32)
    costab = np.tile(np.cos(ang).astype(np.float32).T, (4, 1))  # (128, L)
    sintab = np.tile(np.sin(ang).astype(np.float32).T, (4, 1))

    import ml_dtypes
    ev, od = np.arange(0, HD, 2), np.arange(1, HD, 2)
    wvec = np.empty((128, 4), np.float32)
    for hh in range(HPG):
        r = slice(32 * hh, 32 * hh + 32)
        wvec[r, 0] = 1.0 / (HD * q_scale[ev] ** 2)
        wvec[r, 1] = 1.0 / (HD * q_scale[od] ** 2)
        wvec[r, 2] = 1.0 / (k_scale[ev] ** 2)
        wvec[r, 3] = 1.0 / (k_scale[od] ** 2)

    in_maps = []
    for c in range(8):
        b, g = c // 4, c % 4
        wqk = np.empty((D, 512), np.float32)
        for hh in range(HPG):
            gh = g * HPG + hh
            wq = Wqkv[gh * HD:(gh + 1) * HD, :] * q_scale[:, None]
            wk = Wqkv[D + gh * HD:D + (gh + 1) * HD, :] * k_scale[:, None]
            wqk[:, 0 + 32 * hh:32 + 32 * hh] = wq[ev].T
            wqk[:, 128 + 32 * hh:160 + 32 * hh] = wq[od].T
            wqk[:, 256 + 32 * hh:288 + 32 * hh] = wk[ev].T
            wqk[:, 384 + 32 * hh:416 + 32 * hh] = wk[od].T
        wv = np.ascontiguousarray(
            Wqkv[2 * D + g * CPG:2 * D + (g + 1) * CPG, :].T).astype(ml_dtypes.bfloat16)
        wo = np.ascontiguousarray(Wout[:, g * CPG:(g + 1) * CPG].T)
        in_maps.append({
            "xT": np.ascontiguousarray(x[b].T).astype(ml_dtypes.bfloat16),
            "wqk": wqk.astype(ml_dtypes.bfloat16), "wv": wv,
            "wo": wo.astype(ml_dtypes.bfloat16),
            "wvec": wvec.astype(ml_dtypes.bfloat16),
            "costab": costab, "sintab": sintab,
        })
    return in_maps


def kernel(x, Wqkv, Wout, q_scale, k_scale, T=None, N_p=None):
    assert int(T) == 8 and int(N_p) == 256
    if "nc" not in _CACHE:
        _CACHE["nc"] = _build_nc()
    nc = _CACHE["nc"]
    in_maps = _host_prep(x, Wqkv, Wout, q_scale, k_scale)
    trace = bool(int(os.environ.get("KERNEL_TRACE", "0")))
    res = run_bass_kernel_spmd(nc, in_maps, core_ids=list(range(8)), trace=trace)
    _CACHE["last_exec_time_ns"] = res.exec_time_ns
    outp = np.zeros((B, L, D), np.float32)
    for c in range(8):
        outp[c // 4] += res.results[c]["out"]
    return outp


if __name__ == "__main__":
    rng = np.random.default_rng(0)
    x = rng.standard_normal((B, L, D), dtype=np.float32)
    Wqkv = rng.standard_normal((3 * D, D), dtype=np.float32) * 0.02
    Wout = rng.standard_normal((D, D), dtype=np.float32) * 0.02
    o = kernel(x, Wqkv, Wout, np.ones(HD, np.float32), np.ones(HD, np.float32),
               8, 256)
    print("out", o.shape, o.dtype, float(np.abs(o).mean()))



# revision 16
# speedup vs baseline: 1.5068x; 1.5068x over previous
"""Block-causal attention Trainium2 kernel (8 NeuronCores).

Sharding: core c = b*4 + g handles batch b (of 2) and head-group g (4 of 16
heads). Each core computes the qkv projection, rmsnorm + 2-D RoPE,
block-causal attention and a partial output projection for its 256 channels;
the host sums the 4 per-group partials per batch.

v2 design notes (vs v1): the PE HAM clock gate runs the array at 1.2 GHz
unless it sees ~3.4us of sustained activity, so the kernel is structured to
keep the tensor engine dense end-to-end:
  * rmsnorm sums use one block-diagonal [128,8] weights matmul per component
    (4 matmuls/l-chunk instead of 16).
  * r_q / r_k / softmax-denominator broadcasts across partitions are done
    with tiny ones-matrix matmuls on the PE (weights [4,128]/[2,128]) instead
    of DRAM round-trips through a scratch buffer + gpsimd broadcast DMA.
  * softmax normalization happens per frame-pair right after its attention
    group, and the output projection for frame-pair fp-1 is interleaved into
    frame-pair fp's attention block, so there is no serial tail.
  * the Scalar (Activation) engine runs Exp only during attention (the two
    rsqrt batches happen before the first exp; no activation-table thrash).
  * QPl/KPl shuffle DMAs are issued from the scalar queue (lc 0,1: before any
    exp) and the gpsimd queue (lc 2,3).
Matmuls run in bf16 (the fp32r path lowers to slow 2-pass fp32 on this HW).

On-chip layouts (per core):
  Q^T/K^T: feature-on-partition tiles QR/QI/KR/KI [128, 2048]; row 32*hh+j
    <-> head hh, complex pair j (R = even orig dim 2j, I = odd 2j+1).
  V: v_blk [128, 16, 4, 65]: l-tile lt, head h, 64 features + ones col 64 so
    the softmax denominator falls out of the M=65 PV matmul.
  Scores: S^T [keys=128, q] per (head, frame-pair, ktile); block-causal means
    frame t only attends keys < 256*(t+1) -- no mask tensor anywhere.
  exp() needs no max-subtraction (|scores| <= 8 after rmsnorm).
"""

import os
import numpy as np

import concourse.bass as bass
import concourse.mybir as mybir
import concourse.tile as tile
from concourse import bacc
from concourse.bass_utils import run_bass_kernel_spmd

F32 = mybir.dt.float32
BF16 = mybir.dt.bfloat16
AF = mybir.ActivationFunctionType
MUL = mybir.AluOpType.mult
ADD = mybir.AluOpType.add
SUB = mybir.AluOpType.subtract

B, T, NP, D, H = 2, 8, 256, 1024, 16
L = T * NP            # 2048
HD = 64               # head dim
HPG = 4               # heads per group (4 groups x 2 batches = 8 cores)
CPG = HPG * HD        # 256 channels per group
NDT = D // 128        # 8 d-tiles
NLC = L // 512        # 4 l-chunks
NLT = L // 128        # 16 l-tiles
EPS = 1e-6

_CACHE = {}


def _emit(nc, tc, ctx, xT, wqk, wv, wo, wvec2, emat, bias8, costab, sintab,
          out):
    sing = ctx.enter_context(tc.tile_pool(name="sing", bufs=1))
    xp = ctx.enter_context(tc.tile_pool(name="xp", bufs=16))
    tmp = ctx.enter_context(tc.tile_pool(name="tmp", bufs=2))
    sqp = ctx.enter_context(tc.tile_pool(name="sqp", bufs=4))
    ptp = ctx.enter_context(tc.tile_pool(name="ptp", bufs=8))
    osb = ctx.enter_context(tc.tile_pool(name="osb", bufs=3))
    dnp = ctx.enter_context(tc.tile_pool(name="dnp", bufs=2))
    # PSUM: 3-bank general pool + 5-bank attention pool = 8 banks
    pps = ctx.enter_context(tc.tile_pool(name="pps", bufs=3, space="PSUM"))
    pst = ctx.enter_context(tc.tile_pool(name="pst", bufs=5, space="PSUM"))

    # ---- persistent SBUF; DMA order matters for startup latency ----
    # sync queue: wqk first, then per-lc x tiles (emitted in the lc loop)
    wqk_sb = sing.tile([128, NDT, 512], BF16)
    nc.sync.dma_start(out=wqk_sb[:], in_=wqk.rearrange("(t p) o -> p t o", p=128))
    # gpsimd queue: small tables, then wv (needed in A0), wo last
    wvec2_sb = sing.tile([128, 4, 8], BF16)
    nc.gpsimd.dma_start(out=wvec2_sb[:], in_=wvec2[:])
    emat_sb = sing.tile([64, 4, 128], BF16)
    nc.gpsimd.dma_start(out=emat_sb[:], in_=emat[:])
    wv_sb = sing.tile([128, NDT, CPG], BF16)
    nc.gpsimd.dma_start(out=wv_sb[:], in_=wv.rearrange("(t p) o -> p t o", p=128))
    wo_sb = sing.tile([128, 2, D], BF16)
    nc.gpsimd.dma_start(out=wo_sb[:], in_=wo.rearrange("(t p) o -> p t o", p=128))
    # scalar queue: rope tables (first needed in B0, ~20us in)
    cos_sb = sing.tile([128, L], BF16)
    nc.scalar.dma_start(out=cos_sb[:], in_=costab[:])
    sin_sb = sing.tile([128, L], BF16)
    nc.scalar.dma_start(out=sin_sb[:], in_=sintab[:])

    qk_sb = [sing.tile([128, L], BF16, name=f"qk{i}") for i in range(4)]
    rope_sb = [sing.tile([128, L], BF16, name=f"rope{i}") for i in range(4)]
    v_blk = sing.tile([128, NLT, HPG, 65], BF16)
    att_sb = [sing.tile([128, L], BF16, name=f"att{i}") for i in range(2)]
    ones_v = sing.tile([128, NLT, HPG, 1], F32)
    nc.vector.memset(ones_v[:], 1.0)
    nc.vector.tensor_copy(v_blk[:, :, :, 64:65], ones_v[:])

    bias8_sb = sing.tile([8, 1], F32)
    nc.gpsimd.dma_start(out=bias8_sb[:], in_=bias8[:])

    # rms sums (rows 0-3: q-head means; rows 4-7: k-head sums) and their
    # rsqrt as one [8, L] tile (engine partition offsets must be 32-aligned,
    # so the q/k split happens in the 8-row broadcast weights instead)
    rs_sb = sing.tile([8, L], F32)
    rq8_sb = sing.tile([8, L], BF16)
    # denominator staging rows at partitions 0 and 32; the other rows stay
    # 1.0 so the 64-row broadcast matmul contracts finite values
    dn64 = sing.tile([64, 512], F32)
    nc.vector.memset(dn64[:], 1.0)
    # per-lc shuffled, per-head-contiguous q/k tiles; K pre-scaled by 0.125*r_k
    QPl = [[sing.tile([128, 512], BF16, name=f"qp{i}_{c}") for c in range(NLC)]
           for i in range(2)]
    KPl = [[sing.tile([128, 512], BF16, name=f"kp{i}_{c}") for c in range(NLC)]
           for i in range(2)]

    # ---- phase A: projections + rms-sum matmuls, per l-chunk ----
    def phase_a(lc):
        ls = slice(lc * 512, (lc + 1) * 512)
        xt = []
        for dt in range(NDT):
            x1 = xp.tile([128, 512], BF16, name=f"xt{dt}", tag="xt")
            nc.sync.dma_start(out=x1[:], in_=xT[dt * 128:(dt + 1) * 128, ls])
            xt.append(x1)
        sqs = []
        for ot in range(4):                 # QR, QI, KR, KI
            ps = pps.tile([128, 512], F32, name="qkps", tag="ps")
            for dt in range(NDT):
                nc.tensor.matmul(ps[:], wqk_sb[:, dt, ot * 128:(ot + 1) * 128],
                                 xt[dt][:], start=(dt == 0), stop=(dt == NDT - 1))
            nc.vector.tensor_copy(qk_sb[ot][:, ls], ps[:])
            sq = sqp.tile([128, 512], BF16, tag="sq")
            nc.vector.tensor_tensor(sq[:], qk_sb[ot][:, ls],
                                    qk_sb[ot][:, ls], MUL)
            sqs.append(sq)
        # block-diagonal rms sums: rs[8, 512] accumulates all 4 components
        rs = pst.tile([8, 512], F32, name="rs", tag="st")
        for ot in range(4):
            nc.tensor.matmul(rs[:], wvec2_sb[:, ot, :], sqs[ot][:],
                             start=(ot == 0), stop=(ot == 3),
                             skip_group_check=True)
        nc.vector.tensor_copy(rs_sb[:, ls], rs[:])
        # V projection: l on partitions, ones col for the denominator
        for ls4 in range(4):
            lt = lc * 4 + ls4
            ps = pps.tile([128, CPG], F32, name="vps", tag="ps")
            for dt in range(NDT):
                nc.tensor.matmul(ps[:], xt[dt][:, ls4 * 128:(ls4 + 1) * 128],
                                 wv_sb[:, dt, :], start=(dt == 0),
                                 stop=(dt == NDT - 1))
            for h in range(HPG):
                nc.vector.tensor_copy(v_blk[:, lt, h, 0:64],
                                      ps[:, h * 64:(h + 1) * 64])

    # ---- rsqrt batch for a pair of l-chunks (Scalar sqrt + DVE recip) ----
    def rsqrt_batch(lcpair):
        bs = slice(lcpair * 1024, (lcpair + 1) * 1024)
        nc.scalar.activation(rs_sb[:, bs], rs_sb[:, bs], AF.Sqrt,
                             bias=bias8_sb[:])
        nc.vector.reciprocal_approx_fast(out=rs_sb[:, bs], in_=rs_sb[:, bs])
        nc.vector.tensor_copy(rq8_sb[:, bs], rs_sb[:, bs])

    # ---- phase B: PE broadcast of r, rope, shuffle, k-scale, per l-chunk --
    def phase_b(lc):
        ls = slice(lc * 512, (lc + 1) * 512)
        # broadcast r_q across each head's 32 pair-rows (PE ones-matmul)
        rqf = pps.tile([128, 512], F32, name="rqf", tag="ps")
        nc.tensor.matmul(rqf[:], emat_sb[0:8, 0, :], rq8_sb[:, ls],
                         start=True, stop=True, skip_group_check=True)
        rkf = []
        for hp in range(2):
            rk1 = pst.tile([128, 512], F32, name=f"rkf{hp}", tag="st")
            nc.tensor.matmul(rk1[:], emat_sb[0:8, 1 + hp, :], rq8_sb[:, ls],
                             start=True, stop=True, skip_group_check=True)
            rkf.append(rk1)
        # rope (+ r_q fold on the q side)
        for base in (0, 2):
            xr, xi = qk_sb[base][:, ls], qk_sb[base + 1][:, ls]
            for comp in range(2):
                t1 = tmp.tile([128, 512], F32, tag="t1")
                t2 = tmp.tile([128, 512], F32, tag="t2")
                ca, cb = (cos_sb, sin_sb) if comp == 0 else (sin_sb, cos_sb)
                nc.vector.tensor_tensor(t1[:], xr, ca[:, ls], MUL)
                nc.vector.tensor_tensor(t2[:], xi, cb[:, ls], MUL)
                op = SUB if comp == 0 else ADD
                dst = rope_sb[base + comp][:, ls]
                if base == 0:
                    t3 = tmp.tile([128, 512], F32, tag="t3")
                    nc.vector.tensor_tensor(t3[:], t1[:], t2[:], op)
                    nc.vector.tensor_tensor(dst, t3[:], rqf[:], MUL)
                else:
                    nc.vector.tensor_tensor(dst, t1[:], t2[:], op)

        # shuffle into per-head contiguous tiles (hwdge queues; scalar queue
        # is free of exp work until attention starts)
        dma_eng = nc.scalar if lc < 2 else nc.gpsimd
        for hp2 in range(2):
            for i2 in range(2):
                h2 = hp2 * 2 + i2
                dma_eng.dma_start(out=QPl[hp2][lc][64 * i2:64 * i2 + 32, :],
                                  in_=rope_sb[0][32 * h2:32 * h2 + 32, ls])
                dma_eng.dma_start(out=QPl[hp2][lc][64 * i2 + 32:64 * i2 + 64, :],
                                  in_=rope_sb[1][32 * h2:32 * h2 + 32, ls])
                dma_eng.dma_start(out=KPl[hp2][lc][64 * i2:64 * i2 + 32, :],
                                  in_=rope_sb[2][32 * h2:32 * h2 + 32, ls])
                dma_eng.dma_start(out=KPl[hp2][lc][64 * i2 + 32:64 * i2 + 64, :],
                                  in_=rope_sb[3][32 * h2:32 * h2 + 32, ls])
        # K pre-scale by 0.125*r_k (rkf is per-head broadcast rows), in place
        for hp2 in range(2):
            nc.vector.tensor_tensor(KPl[hp2][lc][:], KPl[hp2][lc][:],
                                    rkf[hp2][:], MUL)

    # ---- attention group (frame pair, kt-major) + fused normalize ----
    def att_group(hp, fp):
        nkt_sh, nkt_all = 4 * fp + 2, 4 * fp + 4
        pvps = [pst.tile([65, 512], F32, name=f"pv{hp}_{fp}_{i}", tag="st")
                for i in range(2)]
        pend = []

        def flush_pv():
            for kt_, i_, pt_ in pend:
                h_ = hp * 2 + i_
                if kt_ < nkt_sh:
                    nc.tensor.matmul(pvps[i_][:, :], v_blk[:, kt_, h_, :],
                                     pt_[:, 0:512], start=(kt_ == 0),
                                     stop=(kt_ == nkt_all - 1),
                                     skip_group_check=True)
                else:
                    nc.tensor.matmul(pvps[i_][:, 256:512], v_blk[:, kt_, h_, :],
                                     pt_[:, 0:256], start=False,
                                     stop=(kt_ == nkt_all - 1),
                                     skip_group_check=True)
            pend.clear()

        for kt in range(nkt_all):
            lck, kof = kt // 4, (kt % 4) * 128
            ksl = slice(kof, kof + 128)
            shared = kt < nkt_sh
            nq = 512 if shared else 256
            cur = []
            for i in range(2):
                qsrc = (QPl[hp][fp][64 * i:64 * i + 64, :] if shared else
                        QPl[hp][fp][64 * i:64 * i + 64, 256:512])
                st = pst.tile([128, nq], F32, name=f"st{i}{nq}", tag="st")
                nc.tensor.matmul(st[:, :],
                                 KPl[hp][lck][64 * i:64 * i + 64, ksl],
                                 qsrc, start=True, stop=True,
                                 skip_group_check=True)
                pt = ptp.tile([128, nq], BF16, name=f"pt{i}{nq}", tag="pt")
                nc.scalar.activation(pt[:], st[:], AF.Exp)
                cur.append((kt, i, pt))
            if len(pend) >= 4:
                flush_pv()
            pend.extend(cur)
        flush_pv()

        # normalize: reciprocal of the per-query denominator, PE-broadcast
        # across the 64 feature rows, multiply-cast into att_sb
        qs = slice(fp * 512, fp * 512 + 512)
        nc.vector.tensor_copy(dn64[0:1, :], pvps[0][64:65, :])
        nc.vector.tensor_copy(dn64[32:33, :], pvps[1][64:65, :])
        nc.vector.reciprocal_approx_fast(out=dn64[:], in_=dn64[:])
        dni = dnp.tile([64, 512], BF16, tag="dni")
        nc.vector.tensor_copy(dni[:], dn64[:])
        rinv_ps = pps.tile([128, 512], F32, name="rinv", tag="ps")
        nc.tensor.matmul(rinv_ps[:], emat_sb[:, 3, :], dni[:],
                         start=True, stop=True, skip_group_check=True)
        rinv = tmp.tile([128, 512], F32, tag="rinv_sb")
        nc.vector.tensor_copy(rinv[:], rinv_ps[:])
        for i in range(2):
            nc.vector.tensor_tensor(att_sb[hp][64 * i:64 * i + 64, qs],
                                    pvps[i][0:64, :],
                                    rinv[64 * i:64 * i + 64, :], MUL)

    # ---- output projection for one frame pair (4 l-tiles) ----
    def out_proj(fp):
        for ls4 in range(4):
            lt = fp * 4 + ls4
            lsl = slice(lt * 128, (lt + 1) * 128)
            pso = [pps.tile([128, 512], F32, name=f"op{oc}", tag="ps")
                   for oc in range(2)]
            for ct in range(2):
                for oc in range(2):
                    nc.tensor.matmul(pso[oc][:], att_sb[ct][:, lsl],
                                     wo_sb[:, ct, oc * 512:(oc + 1) * 512],
                                     start=(ct == 0), stop=(ct == 1))
            for oc in range(2):
                ob = osb.tile([128, 512], F32, tag="ob")
                if (ls4 + oc) % 2 == 0:
                    nc.vector.tensor_copy(ob[:], pso[oc][:])
                else:
                    nc.scalar.activation(ob[:], pso[oc][:], AF.Copy)
                nc.sync.dma_start(out=out[lsl, oc * 512:(oc + 1) * 512],
                                  in_=ob[:])

    # ---- schedule ----
    phase_a(0)
    phase_a(1)
    rsqrt_batch(0)
    phase_a(2)
    phase_b(0)
    phase_b(1)
    phase_a(3)
    rsqrt_batch(1)
    phase_b(2)
    phase_b(3)
    for fp in range(NLC):
        att_group(0, fp)
        att_group(1, fp)
        if fp >= 1:
            out_proj(fp - 1)
    out_proj(3)


def _build_nc():
    import contextlib
    nc = bacc.Bacc("TRN2", target_bir_lowering=False, debug=False, num_devices=8)
    xT = nc.dram_tensor("xT", (D, L), BF16, kind="ExternalInput")
    wqk = nc.dram_tensor("wqk", (D, 512), BF16, kind="ExternalInput")
    wv = nc.dram_tensor("wv", (D, CPG), BF16, kind="ExternalInput")
    wo = nc.dram_tensor("wo", (CPG, D), BF16, kind="ExternalInput")
    wvec2 = nc.dram_tensor("wvec2", (128, 4, 8), BF16, kind="ExternalInput")
    emat = nc.dram_tensor("emat", (64, 4, 128), BF16, kind="ExternalInput")
    bias8 = nc.dram_tensor("bias8", (8, 1), F32, kind="ExternalInput")
    costab = nc.dram_tensor("costab", (128, L), BF16, kind="ExternalInput")
    sintab = nc.dram_tensor("sintab", (128, L), BF16, kind="ExternalInput")
    out = nc.dram_tensor("out", (L, D), F32, kind="ExternalOutput")

    with tile.TileContext(nc) as tc, contextlib.ExitStack() as ctx:
        _emit(nc, tc, ctx, xT.ap(), wqk.ap(), wv.ap(), wo.ap(), wvec2.ap(),
              emat.ap(), bias8.ap(), costab.ap(), sintab.ap(), out.ap())
    nc.compile()
    return nc


def _host_prep(x, Wqkv, Wout, q_scale, k_scale):
    x = np.asarray(x, np.float32)
    Wqkv = np.asarray(Wqkv, np.float32)
    Wout = np.asarray(Wout, np.float32)
    q_scale = np.asarray(q_scale, np.float32)
    k_scale = np.asarray(k_scale, np.float32)

    quarter = HD // 4  # 16
    inv = 1.0 / (10000.0 ** (np.arange(quarter, dtype=np.float64) / quarter))
    tt = np.repeat(np.arange(T), NP).astype(np.float64)
    pp = np.tile(np.arange(NP), T).astype(np.float64)
    ang = np.concatenate([tt[:, None] * inv[None, :], pp[:, None] * inv[None, :]],
                         axis=1)  # (L, 32)

    import ml_dtypes
    costab = np.tile(np.cos(ang).astype(np.float32).T, (4, 1)).astype(
        ml_dtypes.bfloat16)  # (128, L)
    sintab = np.tile(np.sin(ang).astype(np.float32).T, (4, 1)).astype(
        ml_dtypes.bfloat16)

    ev, od = np.arange(0, HD, 2), np.arange(1, HD, 2)
    # block-diagonal rms weights: [128, ot(QR,QI,KR,KI), 8]
    wvec2 = np.zeros((128, 4, 8), np.float32)
    for hh in range(HPG):
        r = slice(32 * hh, 32 * hh + 32)
        wvec2[r, 0, hh] = 1.0 / (HD * q_scale[ev] ** 2)
        wvec2[r, 1, hh] = 1.0 / (HD * q_scale[od] ** 2)
        wvec2[r, 2, 4 + hh] = 1.0 / (k_scale[ev] ** 2)
        wvec2[r, 3, 4 + hh] = 1.0 / (k_scale[od] ** 2)

    # ones matrices for PE partition-broadcast matmuls (zero-padded rows so
    # every rhs can start at partition 0)
    emat = np.zeros((64, 4, 128), np.float32)
    for hh in range(HPG):
        emat[hh, 0, 32 * hh:32 * hh + 32] = 1.0    # r_q: head hh -> 32 rows
    for hp in range(2):
        for i in range(2):
            emat[4 + 2 * hp + i, 1 + hp, 64 * i:64 * i + 64] = 1.0  # r_k
    emat[0, 3, 0:64] = 1.0                         # denominator head-pair 0
    emat[32, 3, 64:128] = 1.0                      # denominator head-pair 1

    bias8 = np.full((8, 1), EPS, np.float32)
    bias8[4:8] = 64.0 * EPS

    in_maps = []
    for c in range(8):
        b, g = c // 4, c % 4
        wqk = np.empty((D, 512), np.float32)
        for hh in range(HPG):
            gh = g * HPG + hh
            wq = Wqkv[gh * HD:(gh + 1) * HD, :] * q_scale[:, None]
            wk = Wqkv[D + gh * HD:D + (gh + 1) * HD, :] * k_scale[:, None]
            wqk[:, 0 + 32 * hh:32 + 32 * hh] = wq[ev].T
            wqk[:, 128 + 32 * hh:160 + 32 * hh] = wq[od].T
            wqk[:, 256 + 32 * hh:288 + 32 * hh] = wk[ev].T
            wqk[:, 384 + 32 * hh:416 + 32 * hh] = wk[od].T
        wv = np.ascontiguousarray(
            Wqkv[2 * D + g * CPG:2 * D + (g + 1) * CPG, :].T).astype(ml_dtypes.bfloat16)
        wo = np.ascontiguousarray(Wout[:, g * CPG:(g + 1) * CPG].T)
        in_maps.append({
            "xT": np.ascontiguousarray(x[b].T).astype(ml_dtypes.bfloat16),
            "wqk": wqk.astype(ml_dtypes.bfloat16), "wv": wv,
            "wo": wo.astype(ml_dtypes.bfloat16),
            "wvec2": wvec2.astype(ml_dtypes.bfloat16),
            "emat": emat.astype(ml_dtypes.bfloat16),
            "bias8": bias8,
            "costab": costab, "sintab": sintab,
        })
    return in_maps


def kernel(x, Wqkv, Wout, q_scale, k_scale, T=None, N_p=None):
    assert int(T) == 8 and int(N_p) == 256
    if "nc" not in _CACHE:
        _CACHE["nc"] = _build_nc()
    nc = _CACHE["nc"]
    in_maps = _host_prep(x, Wqkv, Wout, q_scale, k_scale)
    trace = bool(int(os.environ.get("KERNEL_TRACE", "0")))
    res = run_bass_kernel_spmd(nc, in_maps, core_ids=list(range(8)), trace=trace)
    _CACHE["last_exec_time_ns"] = res.exec_time_ns
    outp = np.zeros((B, L, D), np.float32)
    for c in range(8):
        outp[c // 4] += res.results[c]["out"]
    return outp


if __name__ == "__main__":
    rng = np.random.default_rng(0)
    x = rng.standard_normal((B, L, D), dtype=np.float32)
    Wqkv = rng.standard_normal((3 * D, D), dtype=np.float32) * 0.02
    Wout = rng.standard_normal((D, D), dtype=np.float32) * 0.02
    o = kernel(x, Wqkv, Wout, np.ones(HD, np.float32), np.ones(HD, np.float32),
               8, 256)
    print("out", o.shape, o.dtype, float(np.abs(o).mean()))
